# revision 1
# baseline (speedup 1.0000x reference)
"""Trainium2 Bass kernel for CachedMultiHeadedAttention (tensor-parallel over heads).

Sharding: 8 cores x 4 heads. Each core computes Q projection + attention for
its 4 heads, then a partial output projection against its 512 rows of Wo.
Host sums the 8 partial outputs (the "all-reduce" done at unshard time) and
adds bo.

Device-side layouts are chosen so NO on-chip transposes are needed:
  - x is passed pre-transposed (xT [D, S]) so contraction dims land on
    SBUF partitions for every matmul.
  - k_cache is passed pre-transposed per head (kT [DK, pos]).
  - The reference's softmax quirk (softmax over the QUERY axis) maps to
    scores^T tiles [l_part, s_free]: one fused ACT pass does exp + row-sum.
    The 1/sum normalization is folded into V rows (8x less data than the
    weight matrix).
Precision: streamed operands (x, Wq, k/v caches, Wo, qT, ctxT) are f16
(10 mantissa bits, ~5e-4 relative — full PE rate and half the DMA bytes of
f32r); softmax weights and scaled V run as float32r (full PE rate, unlike
plain fp32's 1/4 rate); all accumulation is f32 in PSUM, and the 8 partial
outputs are summed on the host in f64. The rank-1 k_new/v_new projections
run in bf16 — they only affect one of the 4096 cache rows. Measured
end-to-end relative error: ~6e-4.

Scheduling notes (cost-model-profiled):
  - Each dma_start costs ~625ns on the single serialized HWDGE queue, so
    DMAs are consolidated into ~130 large transfers (a naive version with
    557 DMAs spent 348us in HWDGE alone).
  - Engine queues execute in order, so the ACT-bound softmax loops carry
    "ride" work: the next head's Q-projection matmuls and (in head 0) the
    kv_new projections are emitted inside the S loop, paced per l-tile, with
    ctx matmuls lagged one iteration behind the exp that feeds them.
  - PSUM is exactly 8 banks: scores 2x[128,1024] (4) + ctx [128,1024] (2) +
    single-bank two-pass Q and kv_new accumulators (1+1).
"""

import math

import numpy as np
import ml_dtypes

import concourse.bass as bass
import concourse.mybir as mybir
import concourse.tile as tile
from concourse import bacc
from concourse.bass_utils import run_bass_kernel_spmd

F32 = mybir.dt.float32
F32R = mybir.dt.float32r
BF16 = mybir.dt.bfloat16
F16 = mybir.dt.float16
AF = mybir.ActivationFunctionType

H, D, DK, S = 32, 4096, 128, 1024
NCORES = 8
HP = H // NCORES          # heads per core
DC = D // 128             # contraction chunks for d_model


def build(pos: int):
    L = pos + 1
    LC = (L + 127) // 128          # number of 128-wide l tiles
    LG = (LC + 7) // 8             # l-tile groups of 8 (1024 l per group)
    INV = 1.0 / math.sqrt(DK)

    nc = bacc.Bacc("TRN2", target_bir_lowering=False, debug=False,
                   num_devices=NCORES)

    xT_d = nc.dram_tensor("xT", [D, S], F16, kind="ExternalInput").ap()
    wq_d = nc.dram_tensor("wq", [HP, D, DK], F16, kind="ExternalInput").ap()
    wkv_d = nc.dram_tensor("wkv", [D, 2 * HP * DK], BF16, kind="ExternalInput").ap()
    xl_d = nc.dram_tensor("xl", [128, DC], BF16, kind="ExternalInput").ap()
    bq_d = nc.dram_tensor("bq", [HP, DK, 1], F32, kind="ExternalInput").ap()
    bkv_d = nc.dram_tensor("bkv", [1, 2 * HP * DK], F32, kind="ExternalInput").ap()
    kT_d = nc.dram_tensor("kT", [HP, DK, pos], F16, kind="ExternalInput").ap()
    v_d = nc.dram_tensor("v", [HP, pos, DK], F16, kind="ExternalInput").ap()
    wo_d = nc.dram_tensor("wo", [HP * DK, D], F16, kind="ExternalInput").ap()
    out_d = nc.dram_tensor("out", [S, D], F16, kind="ExternalOutput").ap()

    with tile.TileContext(nc) as tc:
        # Pools are released LIFO; ctxT survives into the output projection,
        # so it sits at the bottom of the SBUF pool stack.
        ctxT_pool = tc.alloc_tile_pool(name="ctxT", bufs=1)
        wo_pool = tc.alloc_tile_pool(name="wop", bufs=1)
        stage_pool = tc.alloc_tile_pool(name="stagep", bufs=1)
        xT_pool = tc.alloc_tile_pool(name="xT", bufs=1)
        qT_pool = tc.alloc_tile_pool(name="qT", bufs=2)
        small = tc.alloc_tile_pool(name="smallp", bufs=1)
        wq_pool = tc.alloc_tile_pool(name="wqp", bufs=8)
        wkv_pool = tc.alloc_tile_pool(name="wkvp", bufs=3)
        kt_pool = tc.alloc_tile_pool(name="ktp", bufs=3)
        v_pool = tc.alloc_tile_pool(name="vp", bufs=3)
        wt_pool = tc.alloc_tile_pool(name="wtp", bufs=4)
        vs_pool = tc.alloc_tile_pool(name="vsp", bufs=4)
        ss_pool = tc.alloc_tile_pool(name="ssp", bufs=8)

        # PSUM budget (8 banks): psq 1 + kv 1 + pss 4 + psc 2.
        # Q projections and the kv_new projections run in TWO s-half /
        # k-v passes so their accumulators are single-bank.
        psq = tc.alloc_tile_pool(name="psq", bufs=1, space="PSUM")
        kv_pool = tc.alloc_tile_pool(name="kvp", bufs=1, space="PSUM")
        pss = tc.alloc_tile_pool(name="pss", bufs=2, space="PSUM")
        psc = tc.alloc_tile_pool(name="psc", bufs=1, space="PSUM")

        ctxTs = [ctxT_pool.tile([128, S], F16, name=f"cT{h}", tag=f"cT{h}")
                 for h in range(HP)]

        # small constants first (tiny DMAs, ahead of the big streams)
        kvrow = small.tile([1, 2 * HP * DK], F16, name="kvrow", tag="kvrow")
        bkv_t = small.tile([1, 2 * HP * DK], F32, name="bkvt", tag="bkvt")
        nc.sync.dma_start(bkv_t[:], bkv_d[:])
        xl_t = small.tile([128, DC], BF16, name="xlt", tag="xlt")
        nc.sync.dma_start(xl_t[:], xl_d[:])

        # resident xT tiles (8 big tiles of 4 chunks), interleaved with head
        # 0's Q weight groups so the first Q matmuls start after ~2.5MB, not
        # after the full 17MB of x.
        xbig = []
        wq0_groups = []
        for gx in range(DC // 4):
            wqt = wq_pool.tile([128, 4 * DK], F16, name=f"wq0_{gx}", tag="wq")
            nc.sync.dma_start(
                wqt[:], wq_d[0, gx * 512:(gx + 1) * 512, :].rearrange(
                    "(i p) k -> p i k", p=128))
            wq0_groups.append(wqt)
            xt = xT_pool.tile([128, 4 * S], F16, name=f"xt{gx}", tag=f"xt{gx}")
            nc.sync.dma_start(
                xt[:], xT_d[gx * 512:(gx + 1) * 512, :].rearrange(
                    "(i p) s -> p i s", p=128))
            xbig.append(xt)

        def xsl(c, lo, sz):
            return xbig[c // 4][:, (c % 4) * S + lo:(c % 4) * S + lo + sz]

        def emit_wq_dma(h, gw, tag="wq"):
            wqt = wq_pool.tile([128, 4 * DK], F16,
                               name=f"wq{h}_{gw}", tag=tag)
            nc.sync.dma_start(
                wqt[:], wq_d[h, gw * 512:(gw + 1) * 512, :].rearrange(
                    "(i p) k -> p i k", p=128))
            return wqt

        def q_half_mm(psq_t, wqt, c, half):
            lhs = wqt[:, (c % 4) * DK:(c % 4 + 1) * DK]
            nc.tensor.matmul(psq_t[:], lhs, xsl(c, half * 512, 512),
                             start=(c == 0), stop=(c == DC - 1))

        def q_half_add(h, qT_t, psq_t, half, bq_t):
            nc.vector.tensor_scalar_add(qT_t[:, half * 512:(half + 1) * 512],
                                        psq_t[:], bq_t[:])

        kv_cur = {}

        def kv_mm(kv_t, c, which):
            # which: 0 = k_new, 1 = v_new. Weight chunks are DMA'd two at a
            # time — each dma_start costs ~625ns of serialized HWDGE.
            if c % 4 == 0:
                wkvt = wkv_pool.tile([128, 4 * HP * DK], BF16,
                                     name=f"wkv{which}_{c}", tag="wkv")
                nc.sync.dma_start(
                    wkvt[:], wkv_d[c * 128:(c + 4) * 128,
                                   which * HP * DK:(which + 1) * HP * DK]
                    .rearrange("(i p) k -> p i k", p=128))
                kv_cur["t"] = wkvt
            wkvt = kv_cur["t"]
            nc.tensor.matmul(kv_t[0:1, :], xl_t[:, c:c + 1],
                             wkvt[:, (c % 4) * HP * DK:(c % 4 + 1) * HP * DK],
                             start=(c == 0), stop=(c == DC - 1))

        def kv_add(kv_t, which):
            nc.vector.tensor_add(
                kvrow[0:1, which * HP * DK:(which + 1) * HP * DK], kv_t[:],
                bkv_t[0:1, which * HP * DK:(which + 1) * HP * DK])

        def load_group(h, g):
            """Cache-only loads of l-group g (the new-entry writes are
            emitted separately, after kvrow's writes in trace order)."""
            g0 = g * 1024
            gl = min(1024, L - g0)            # valid l in group
            gc = max(0, min(1024, pos - g0))  # of which from cache
            kt8 = kt_pool.tile([128, 1024], F16, name=f"kt{h}_{g}", tag="kt")
            if gc > 0:
                nc.sync.dma_start(kt8[:, 0:gc], kT_d[h, :, g0:g0 + gc])
            if gl < 1024:
                nc.vector.memset(kt8[:, gl:1024], 0.0)
            v8 = v_pool.tile([128, 1024], F16, name=f"v{h}_{g}", tag="v")
            if gl < 1024:
                # zero whole padded chunks first (full partition range — DVE
                # requires 32-aligned partition bases); valid rows are DMA'd
                # over the zeros below.
                nc.vector.memset(v8[:, (gl // 128) * 128:1024], 0.0)
            fc = gc // 128
            if fc > 0:
                nc.sync.dma_start(
                    v8[:, 0:fc * 128],
                    v_d[h, g0:g0 + fc * 128, :].rearrange(
                        "(i p) k -> p i k", p=128))
            rem = gc - fc * 128
            if rem > 0:
                nc.sync.dma_start(v8[0:rem, fc * 128:(fc + 1) * 128],
                                  v_d[h, g0 + fc * 128:g0 + gc, :])
            return kt8, v8

        def new_entry_writes(h, kt8, v8):
            # column/row for l == pos from the biased kvrow
            gp = pos % 1024
            nc.sync.dma_start(kt8[:, gp:gp + 1],
                              kvrow[0:1, h * DK:(h + 1) * DK])
            nc.sync.dma_start(
                v8[gp % 128:gp % 128 + 1, (gp // 128) * 128:(gp // 128 + 1) * 128],
                kvrow[0:1, HP * DK + h * DK:HP * DK + (h + 1) * DK])

        npos_g = pos // 1024            # l-group holding the new entry
        npos_lt = pos // 128            # l-tile index holding the new entry
        # riding is only possible when the S loop is long enough for the
        # 2-instruction-per-lt passes to finish before the new entry is used
        ride_kv = LC >= DC and npos_lt >= 8
        ride_q = LC >= DC

        # ---------- head 0 Q projection (phase A, DMA-paced) ----------
        bq_t = ss_pool.tile([128, 1], F32, name="bq0", tag="bq", bufs=2)
        nc.sync.dma_start(bq_t[:], bq_d[0])
        qT_t = qT_pool.tile([128, S], F16, name="qT0", tag="qT")
        # both s-halves accumulate concurrently (pass B borrows the idle kv
        # bank) so the whole projection rides the x-arrival gaps instead of
        # serializing 6.8us of pass-B matmuls after the stream ends
        psq_a = psq.tile([128, 512], F32, name="psq0_0", tag="psq")
        psq_b = kv_pool.tile([128, 512], F32, name="psq0_1", tag="kv")
        for c in range(DC):
            q_half_mm(psq_a, wq0_groups[c // 4], c, 0)
            q_half_mm(psq_b, wq0_groups[c // 4], c, 1)
        q_half_add(0, qT_t, psq_a, 0, bq_t)
        q_half_add(0, qT_t, psq_b, 1, bq_t)

        if not ride_kv:
            # fallback: dense kv_new before the S loops
            for which in range(2):
                kv_t = kv_pool.tile([1, HP * DK], F32, name=f"kv{which}", tag="kv")
                for c in range(DC):
                    kv_mm(kv_t, c, which)
                kv_add(kv_t, which)

        for h in range(HP):
            # per-lt ride items emitted right after the scores matmuls
            rides = [[] for _ in range(LC)]
            if h + 1 < HP and ride_q:
                bq1 = ss_pool.tile([128, 1], F32, name=f"bq{h+1}", tag="bq",
                                   bufs=2)
                nc.sync.dma_start(bq1[:], bq_d[h + 1])
                qT_next = qT_pool.tile([128, S], F16, name=f"qT{h+1}", tag="qT")
                state = {}

                def mk_q(lt, h1=h + 1, qn=qT_next, bqt=bq1, st=state):
                    def emit():
                        half, c0 = divmod(2 * lt, DC)
                        if c0 == 0 and half == 0:
                            st["wqts"] = {}
                        if c0 == 0:
                            st["psq"] = psq.tile([128, 512], F32,
                                                 name=f"psq{h1}_{half}", tag="psq")
                        for c in (c0, c0 + 1):
                            gw = c // 4
                            if half == 0 and c % 4 == 0:
                                # pass B reuses these resident tiles (8 slots)
                                st["wqts"][gw] = emit_wq_dma(h1, gw)
                            q_half_mm(st["psq"], st["wqts"][gw], c, half)
                        if c0 + 1 == DC - 1:
                            q_half_add(h1, qn, st["psq"], half, bqt)
                    return emit

                for lt in range(DC):
                    rides[lt].append(mk_q(lt))
            if h == 0 and ride_kv:
                # kv_new work items, paced so both passes (and their kvrow
                # writes) are emitted strictly before lt == npos_lt
                kv_work = ([("mm", 0, c) for c in range(DC)] + [("add", 0, 0)]
                           + [("mm", 1, c) for c in range(DC)] + [("add", 1, 0)])
                kvstate = {}

                def kv_emit_one(item, st=kvstate):
                    kind, which, c = item
                    if kind == "add":
                        kv_add(st["kv"], which)
                        return
                    if c == 0:
                        st["kv"] = kv_pool.tile([1, HP * DK], F32,
                                                name=f"kv{which}", tag="kv")
                    kv_mm(st["kv"], c, which)

                n_slots = npos_lt - 1          # ride slots: lt 0..npos_lt-2
                n_pre = max(0, len(kv_work) - 2 * n_slots)
                for item in kv_work[:n_pre]:
                    kv_emit_one(item)
                rest = kv_work[n_pre:]
                for k, item in enumerate(rest):
                    rides[k // 2].append(
                        (lambda it=item: kv_emit_one(it)))

            o_staged = {}
            o_post = []
            if h == HP - 1 and LC >= DC:
                # S_3 has no Q to ride; its psq/kv PSUM banks are dead. Ride
                # the first-3-chunk partials of 16 output tiles there, staged
                # to SBUF; the O phase finishes them with one matmul + add.
                wos = [wo_pool.tile([128, D], F16, name=f"wo{c}", tag=f"wo{c}")
                       for c in range(HP)]

                def mk_wo(c):
                    return lambda: nc.sync.dma_start(
                        wos[c][:], wo_d[c * 128:(c + 1) * 128, :])

                o_tiles = [(s_t, mg) for s_t in (6, 7) for mg in range(D // 512)]
                o_state = {}

                def mk_o(item, st=o_state):
                    t, k = item
                    s_t, mg = o_tiles[t]

                    def emit():
                        if k == 0:
                            pool = kv_pool if t % 2 == 0 else psq
                            st["ps"] = pool.tile(
                                [128, 512], F32, name=f"ops{t}",
                                tag="kv" if t % 2 == 0 else "psq")
                        if k < 3:
                            nc.tensor.matmul(
                                st["ps"][:],
                                ctxTs[k][:, s_t * 128:(s_t + 1) * 128],
                                wos[k][:, mg * 512:(mg + 1) * 512],
                                start=(k == 0), stop=(k == 2))
                        else:
                            sg = stage_pool.tile([128, 512], F16,
                                                 name=f"sg{t}", tag=f"sg{t}")
                            nc.vector.tensor_copy(sg[:], st["ps"][:])
                            o_staged[(s_t, mg)] = sg
                    return emit

                # wo0/wo1 load right after S_3's first K/V group; wo2/wo3
                # trail via the ride slots they're needed in
                o_post.extend([mk_wo(0), mk_wo(1)])
                rides[2].append(mk_wo(2))
                rides[10].append(mk_wo(3))
                o_work = [(t, k) for t in range(len(o_tiles)) for k in range(4)]
                for idx, item in enumerate(o_work):
                    rides[6 + idx // 3].append(mk_o(item))

            psc_t = psc.tile([128, S], F32, name=f"psc{h}", tag="psc")
            cur = load_group(h, 0)
            for fn_ in o_post:
                fn_()
            if not (h == 0 and ride_kv) and npos_g == 0 and npos_lt < LC:
                new_entry_writes(h, *cur)
            nxt = None
            pend = None              # lag-1 ctx: (lt, wt, vst)
            for lt in range(LC):
                g, j = lt // 8, lt % 8
                if j == 0 and g > 0:
                    cur = nxt
                if j == 0 and g + 1 < (LC + 7) // 8:
                    nxt = load_group(h, g + 1)
                    if not (h == 0 and ride_kv) and npos_g == g + 1:
                        new_entry_writes(h, *nxt)
                kt8, v8 = cur
                if h == 0 and ride_kv and lt == npos_lt:
                    # kvrow writes were emitted at lt <= npos_lt - 1
                    new_entry_writes(h, kt8, v8) if npos_g == g else None
                    if npos_g == g + 1 and nxt is not None:
                        new_entry_writes(h, *nxt)

                ps = pss.tile([128, 1024], F32, name=f"ps_{h}_{lt}", tag="pss")
                ksl = kt8[:, j * 128:(j + 1) * 128]
                nc.tensor.matmul(ps[:, 0:512], ksl, qT_t[:, 0:512])
                nc.tensor.matmul(ps[:, 512:1024], ksl, qT_t[:, 512:1024])

                for emit in rides[lt]:
                    emit()

                wt = wt_pool.tile([128, 1024], F32R, name=f"wt_{h}_{lt}", tag="wt")
                ssum = ss_pool.tile([128, 1], F32, name=f"ss_{h}_{lt}", tag="ssum")
                nc.scalar.activation(wt[:], ps[:], AF.Exp, scale=INV, accum_out=ssum[:])
                rec = ss_pool.tile([128, 1], F32, name=f"rc_{h}_{lt}", tag="rec")
                nc.vector.reciprocal(rec[:], ssum[:])
                vst = vs_pool.tile([128, DK], F32R, name=f"vs{h}_{lt}", tag="vs")
                nc.vector.tensor_scalar_mul(vst[:], v8[:, j * 128:(j + 1) * 128], rec[:])

                if pend is not None:
                    plt, pwt, pvst = pend
                    nc.tensor.matmul(psc_t[:, 0:512], pvst[:], pwt[:, 0:512],
                                     start=(plt == 0), stop=False)
                    nc.tensor.matmul(psc_t[:, 512:1024], pvst[:], pwt[:, 512:1024],
                                     start=(plt == 0), stop=False)
                pend = (lt, wt, vst)
            plt, pwt, pvst = pend
            nc.tensor.matmul(psc_t[:, 0:512], pvst[:], pwt[:, 0:512],
                             start=(plt == 0), stop=True)
            nc.tensor.matmul(psc_t[:, 512:1024], pvst[:], pwt[:, 512:1024],
                             start=(plt == 0), stop=True)
            nc.vector.tensor_copy(ctxTs[h][:], psc_t[:])
            if h + 1 < HP and not ride_q:
                # dense fallback Q projection for the next head
                bq1 = ss_pool.tile([128, 1], F32, name=f"bq{h+1}", tag="bq",
                                   bufs=2)
                nc.sync.dma_start(bq1[:], bq_d[h + 1])
                qT_next = qT_pool.tile([128, S], F16, name=f"qT{h+1}", tag="qT")
                wqts_fb = {}
                for half in range(2):
                    psq_t = psq.tile([128, 512], F32,
                                     name=f"psq{h+1}_{half}", tag="psq")
                    for c in range(DC):
                        if half == 0 and c % 4 == 0:
                            wqts_fb[c // 4] = emit_wq_dma(h + 1, c // 4)
                        q_half_mm(psq_t, wqts_fb[c // 4], c, half)
                    q_half_add(h + 1, qT_next, psq_t, half, bq1)
            if h + 1 < HP:
                qT_t = qT_next

        # release attention-phase pools before the output projection (LIFO)
        for p in (psc, pss, kv_pool, psq,
                  ss_pool, vs_pool, wt_pool, v_pool, kt_pool,
                  wkv_pool, wq_pool, small, qT_pool, xT_pool):
            p.release()

        # ---------- output projection: out[s, m] partial ----------
        # Wo fully resident in the space freed by xT; one 16KB-burst output
        # DMA per s-tile.
        ob_pool = tc.alloc_tile_pool(name="obp", bufs=2)
        pso = tc.alloc_tile_pool(name="pso", bufs=4, space="PSUM")
        if not o_staged:
            # fallback path (short sequences): load Wo here
            wos = []
            for c in range(HP):
                wot = wo_pool.tile([128, D], F16, name=f"wo{c}", tag=f"wo{c}")
                nc.sync.dma_start(wot[:], wo_d[c * 128:(c + 1) * 128, :])
                wos.append(wot)
        for s_t in range(S // 128):
            ob = ob_pool.tile([128, D], F16, name=f"ob{s_t}", tag="ob")
            for mg in range(D // 512):
                sg = o_staged.get((s_t, mg))
                pso_t = pso.tile([128, 512], F32, name=f"po{s_t}_{mg}", tag="pso")
                if sg is not None:
                    nc.tensor.matmul(pso_t[:],
                                     ctxTs[HP - 1][:, s_t * 128:(s_t + 1) * 128],
                                     wos[HP - 1][:, mg * 512:(mg + 1) * 512])
                    nc.vector.tensor_add(ob[:, mg * 512:(mg + 1) * 512],
                                         sg[:], pso_t[:])
                else:
                    for c in range(HP):
                        nc.tensor.matmul(pso_t[:],
                                         ctxTs[c][:, s_t * 128:(s_t + 1) * 128],
                                         wos[c][:, mg * 512:(mg + 1) * 512],
                                         start=(c == 0), stop=(c == HP - 1))
                    nc.vector.tensor_copy(ob[:, mg * 512:(mg + 1) * 512], pso_t[:])
            if s_t == S // 128 - 1:
                # stream the final tile's output per mg-pair: the exposed
                # post-compute transfer shrinks to a quarter row-band
                for q in range(8):
                    nc.sync.dma_start(
                        out_d[s_t * 128:(s_t + 1) * 128,
                              q * (D // 8):(q + 1) * (D // 8)],
                        ob[:, q * (D // 8):(q + 1) * (D // 8)])
            else:
                nc.sync.dma_start(out_d[s_t * 128:(s_t + 1) * 128, :], ob[:])
        for p in (pso, ob_pool, stage_pool, wo_pool, ctxT_pool):
            p.release()

    nc.compile()
    return nc


_CACHE = {}
LAST_EXEC_NS = None


def kernel(x, k_cache, v_cache, Wq, bq, Wk, bk, Wv, bv, Wo, bo, pos):
    global LAST_EXEC_NS
    pos = int(pos)

    def f32(a):
        return np.ascontiguousarray(np.asarray(a), dtype=np.float32)

    x = f32(x)
    k_cache, v_cache = f32(k_cache), f32(v_cache)
    Wq, Wk, Wv, Wo = f32(Wq), f32(Wk), f32(Wv), f32(Wo)
    bq, bk, bv, bo = f32(bq), f32(bk), f32(bv), f32(bo)

    xT = np.ascontiguousarray(x[0].T.astype(np.float16))   # [D, S]
    xl = np.ascontiguousarray(
        x[0, -1].reshape(DC, 128).T.astype(ml_dtypes.bfloat16))
    in_maps = []
    for i in range(NCORES):
        hs = slice(i * HP, (i + 1) * HP)
        in_maps.append({
            "xT": xT,
            "wq": np.ascontiguousarray(Wq[hs].astype(np.float16)),
            "wkv": np.ascontiguousarray(np.concatenate([
                Wk[hs].transpose(1, 0, 2).reshape(D, HP * DK),
                Wv[hs].transpose(1, 0, 2).reshape(D, HP * DK)],
                axis=1).astype(ml_dtypes.bfloat16)),
            "xl": xl,
            "bq": np.ascontiguousarray(bq[hs].reshape(HP, DK, 1)),
            "bkv": np.ascontiguousarray(np.concatenate(
                [bk[hs].reshape(-1), bv[hs].reshape(-1)])[None, :]),
            "kT": np.ascontiguousarray(
                k_cache[hs, :pos, :].transpose(0, 2, 1).astype(np.float16)),
            "v": np.ascontiguousarray(v_cache[hs, :pos, :].astype(np.float16)),
            "wo": np.ascontiguousarray(
                Wo[i * HP * DK:(i + 1) * HP * DK].astype(np.float16)),
        })

    if pos not in _CACHE:
        _CACHE[pos] = build(pos)
    nc = _CACHE[pos]

    res = run_bass_kernel_spmd(nc, in_maps, core_ids=list(range(NCORES)))
    LAST_EXEC_NS = res.exec_time_ns

    acc = np.zeros((S, D), np.float64)
    for r in res.results:
        acc += r["out"]
    out = (acc + bo.astype(np.float64)).astype(np.float32)
    return out[None]



# revision 38
# speedup vs baseline: 1.1164x; 1.1164x over previous
"""Trainium2 Bass kernel for CachedMultiHeadedAttention (tensor-parallel over heads).

Sharding: 8 cores x 4 heads. Each core computes Q projection + attention for
its 4 heads, then a partial output projection against its 512 rows of Wo.
Host sums the 8 partial outputs, divides by the fp8 scale product, adds bo.

v2: the Q and output projections run as 3-term e4m3 DoubleRow matmuls
(hi/lo splits, 0.5 cyc/row with 256-wide contraction = 4x fp16 rate):
  q  = xhi@Whi + xlo@Whi + xhi@Wlo        (x*16, Wq*256 host-split)
  out = cthi@Wohi + ctlo@Wohi + cthi@Wolo (ctx*32 device-split, Wo*256 host)
Dropped cross term contributes ~0.07%; measured end-to-end rel err ~1.7e-3.
Scores stay f16 (fp8 would put ~3.7% noise on the logits); softmax weights
and scaled V are f32r (exact, full PE rate). The exp scale folds away the
16*256 operand prescale (INV/4096); the host divides partials by 32*256.

Scheduling (cost-model-profiled):
  - Engine queues execute in order; the ACT-bound softmax loops carry "ride"
    work: the next head's 96 Q DoubleRow matmuls (+ wq hi/lo DMAs) and (in
    head 0) the kv_new projections are emitted inside the S loop, paced per
    l-tile, with ctx matmuls lagged one iteration behind the exp.
  - PSUM 8 banks: scores 2x[128,1024] (4) + ctx [128,1024] (2) + Q / kv_new
    accumulators (1+1).
  - Each dma_start costs ~625ns serialized HWDGE; DMAs are consolidated.
"""

import math

import numpy as np
import ml_dtypes

import concourse.bass as bass
import concourse.mybir as mybir
import concourse.tile as tile
from concourse import bacc
from concourse.bass_utils import run_bass_kernel_spmd

F32 = mybir.dt.float32
F32R = mybir.dt.float32r
BF16 = mybir.dt.bfloat16
F16 = mybir.dt.float16
E4 = mybir.dt.float8e4
AF = mybir.ActivationFunctionType
ALU = mybir.AluOpType
DR = mybir.MatmulPerfMode.DoubleRow
E4NP = ml_dtypes.float8_e4m3

H, D, DK, S = 32, 4096, 128, 1024
NCORES = 8
HP = H // NCORES          # heads per core
DC = D // 128             # contraction chunks for d_model
XS = 16.0                 # x prescale
WS = 256.0                # Wq / Wo prescale
CS = 32.0                 # ctx prescale (device-side split)


def build(pos: int):
    L = pos + 1
    LC = (L + 127) // 128          # number of 128-wide l tiles
    LG = (LC + 7) // 8             # l-tile groups of 8 (1024 l per group)
    INV = 1.0 / math.sqrt(DK)
    QSC = float(INV / (XS * WS))   # exp scale absorbing x/Wq prescale

    nc = bacc.Bacc("TRN2", target_bir_lowering=False, debug=False,
                   num_devices=NCORES)

    # pre-rearranged on host so every DMA descriptor is >= 512B contiguous:
    # x: [128, DC*S] with row p holding chunks c at cols c*S+s (x[c*128+p, s])
    # wq: [HP, 128, DC*DK] with row p holding chunk c at cols c*DK+k
    xhi_d = nc.dram_tensor("xhi", [128, DC * S], E4, kind="ExternalInput").ap()
    xlo_d = nc.dram_tensor("xlo", [128, DC * S], E4, kind="ExternalInput").ap()
    wqh_d = nc.dram_tensor("wqh", [HP, 128, DC * DK], E4, kind="ExternalInput").ap()
    wql_d = nc.dram_tensor("wql", [HP, 128, DC * DK], E4, kind="ExternalInput").ap()
    wkv_d = nc.dram_tensor("wkv", [D, 2 * HP * DK], BF16, kind="ExternalInput").ap()
    xl_d = nc.dram_tensor("xl", [128, DC], BF16, kind="ExternalInput").ap()
    bq_d = nc.dram_tensor("bq", [HP, DK, 1], F32, kind="ExternalInput").ap()
    bkv_d = nc.dram_tensor("bkv", [128, 2 * HP], F32, kind="ExternalInput").ap()
    kT_d = nc.dram_tensor("kT", [HP, DK, pos], F16, kind="ExternalInput").ap()
    # v pre-grouped on host: [h, g, p, i*DK+k] = v[h, g*1024+i*128+p, k],
    # zero-padded past pos (the new-entry row is overwritten on device)
    LGv = (pos + 1024) // 1024
    v_d = nc.dram_tensor("v", [HP, LGv, 128, 1024], F16, kind="ExternalInput").ap()
    # Wo pairs: [pair, 128, 2*D] with head-chunk 2p at cols 0:D, 2p+1 at D:2D
    woh_d = nc.dram_tensor("woh", [HP // 2, 128, 2 * D], E4, kind="ExternalInput").ap()
    wol_d = nc.dram_tensor("wol", [HP // 2, 128, 2 * D], E4, kind="ExternalInput").ap()
    out_d = nc.dram_tensor("out", [S, D], F16, kind="ExternalOutput").ap()

    with tile.TileContext(nc) as tc:
        # Pools are released LIFO; ct (quantized ctx) survives into the
        # output projection, so it sits at the bottom of the SBUF stack.
        ct_pool = tc.alloc_tile_pool(name="ctp", bufs=1)
        wo_pool = tc.alloc_tile_pool(name="wop", bufs=1)
        stage_pool = tc.alloc_tile_pool(name="stgp", bufs=1)
        xT_pool = tc.alloc_tile_pool(name="xT", bufs=1)
        qT_pool = tc.alloc_tile_pool(name="qT", bufs=2)
        small = tc.alloc_tile_pool(name="smallp", bufs=1)
        wq_pool = tc.alloc_tile_pool(name="wqp", bufs=2)
        wkv_pool = tc.alloc_tile_pool(name="wkvp", bufs=2)
        kt_pool = tc.alloc_tile_pool(name="ktp", bufs=3)
        v_pool = tc.alloc_tile_pool(name="vp", bufs=3)
        wt_pool = tc.alloc_tile_pool(name="wtp", bufs=4)
        vs_pool = tc.alloc_tile_pool(name="vsp", bufs=4)
        ss_pool = tc.alloc_tile_pool(name="ssp", bufs=8)

        psq = tc.alloc_tile_pool(name="psq", bufs=1, space="PSUM")
        kv_pool = tc.alloc_tile_pool(name="kvp", bufs=1, space="PSUM")
        pss = tc.alloc_tile_pool(name="pss", bufs=2, space="PSUM")
        psc = tc.alloc_tile_pool(name="psc", bufs=1, space="PSUM")

        # quantized-ctx pair tiles: [128, 2048] = heads (2p, 2p+1) side by side
        cthi = [ct_pool.tile([128, 2 * S], E4, name=f"cth{p}", tag=f"cth{p}")
                for p in range(HP // 2)]
        ctlo = [ct_pool.tile([128, 2 * S], E4, name=f"ctl{p}", tag=f"ctl{p}")
                for p in range(HP // 2)]

        # small constants first (tiny DMAs, ahead of the big streams)
        # kvrow layout: [128 k, which*HP + head] (transposed rank-1 results)
        kvrow = small.tile([128, 2 * HP], F16, name="kvrow", tag="kvrow")
        bkv_t = small.tile([128, 2 * HP], F32, name="bkvt", tag="bkvt")
        nc.sync.dma_start(bkv_t[:], bkv_d[:])
        xl_t = small.tile([128, DC], BF16, name="xlt", tag="xlt")
        nc.sync.dma_start(xl_t[:], xl_d[:])

        # resident x hi/lo tiles (8 big tiles of 4 chunks each per tensor),
        # hi tiles interleaved with head 0's Q weight groups so the first Q
        # matmuls start early; lo tiles follow.
        def emit_wq_dma(h, which):
            """One consolidated DMA for a whole head's Wq hi or lo tensor
            (16 separate dma_starts would cost 10us of serialized HWDGE)."""
            src = wqh_d if which == 0 else wql_d
            wqt = wq_pool.tile([128, DC * DK], E4,
                               name=f"wq{which}_{h}", tag=f"wq{which}")
            nc.sync.dma_start(wqt[:], src[h])
            return wqt

        wq0_hi = emit_wq_dma(0, 0)
        wq0_lo = emit_wq_dma(0, 1)
        xbig_hi, xbig_lo = [], []
        for gx in range(DC // 4):
            xt = xT_pool.tile([128, 4 * S], E4, name=f"xh{gx}", tag=f"xh{gx}")
            nc.sync.dma_start(xt[:], xhi_d[:, gx * 4 * S:(gx + 1) * 4 * S])
            xbig_hi.append(xt)

        def load_xlo():
            for gx in range(DC // 4):
                xt = xT_pool.tile([128, 4 * S], E4, name=f"xl{gx}", tag=f"xl{gx}")
                nc.sync.dma_start(xt[:], xlo_d[:, gx * 4 * S:(gx + 1) * 4 * S])
                xbig_lo.append(xt)

        def x_pair(xbig, pr, half):
            """rhs AP [128, 2, 512] for chunk pair (2pr, 2pr+1), s-half."""
            t = xbig[pr // 2]
            i = (pr % 2) * 2
            a3 = t[:, i * S:(i + 2) * S].rearrange("p (two s) -> p two s", two=2)
            return a3[:, :, half * 512:half * 512 + 512]

        def wq_pair(wqt, pr):
            """lhsT AP [128, 2, 128] for chunk pair (2pr, 2pr+1)."""
            return wqt[:, 2 * pr * DK:(2 * pr + 2) * DK].rearrange(
                "p (two k) -> p two k", two=2)

        def q_mm(psq_t, wqt, xbig, pr, half, start, stop):
            nc.tensor.matmul(psq_t[:], wq_pair(wqt, pr), x_pair(xbig, pr, half),
                             start=start, stop=stop, perf_mode=DR)

        def q_half_add(qT_t, psq_t, half, bq_t):
            nc.vector.tensor_scalar_add(qT_t[:, half * 512:(half + 1) * 512],
                                        psq_t[:], bq_t[:])

        kv_cur = {}

        def kv_mm(kv_t, c, which, b):
            """Transposed rank-1 update: out [128k, 1] per 128-wide k block
            (1 PE cycle each vs 512 for the [1, 512]-out orientation)."""
            if b == 0 and c % 4 == 0:
                wkvt = wkv_pool.tile([128, 4 * HP * DK], BF16,
                                     name=f"wkv{which}_{c}", tag="wkv")
                nc.sync.dma_start(
                    wkvt[:], wkv_d[c * 128:(c + 4) * 128,
                                   which * HP * DK:(which + 1) * HP * DK]
                    .rearrange("(i p) k -> p i k", p=128))
                kv_cur["t"] = wkvt
            wkvt = kv_cur["t"]
            nc.tensor.matmul(
                kv_t[:, which * HP + b:which * HP + b + 1],
                wkvt[:, (c % 4) * HP * DK + b * DK:
                     (c % 4) * HP * DK + (b + 1) * DK],
                xl_t[:, c:c + 1],
                start=(c == 0), stop=(c == DC - 1), skip_group_check=True)

        def kv_add(kv_t):
            nc.vector.tensor_add(kvrow[:], kv_t[:], bkv_t[:])

        def load_group(h, g):
            """Cache-only loads of l-group g."""
            g0 = g * 1024
            gl = min(1024, L - g0)            # valid l in group
            gc = max(0, min(1024, pos - g0))  # of which from cache
            kt8 = kt_pool.tile([128, 1024], F16, name=f"kt{h}_{g}", tag="kt")
            if gc > 0:
                nc.sync.dma_start(kt8[:, 0:gc], kT_d[h, :, g0:g0 + gc])
            if gl < 1024:
                nc.vector.memset(kt8[:, gl:1024], 0.0)
            v8 = v_pool.tile([128, 1024], F16, name=f"v{h}_{g}", tag="v")
            nc.sync.dma_start(v8[:], v_d[h, g])
            return kt8, v8

        def new_entry_writes(h, kt8, v8):
            gp = pos % 1024
            nc.sync.dma_start(kt8[:, gp:gp + 1], kvrow[:, h:h + 1])
            nc.sync.dma_start(
                v8[gp % 128:gp % 128 + 1, (gp // 128) * 128:(gp // 128 + 1) * 128],
                kvrow[:, HP + h:HP + h + 1])

        npos_g = pos // 1024            # l-group holding the new entry
        npos_lt = pos // 128            # l-tile index holding the new entry
        ride_kv = LC >= DC and npos_lt >= 8
        ride_q = LC >= DC

        # ---------- head 0 Q projection (phase A, DMA-paced) ----------
        # Both s-halves accumulate concurrently (pass B borrows the idle kv
        # bank) so the projection rides the x-arrival gaps. Term order:
        # t0 = xhi (x)Whi per pair as xhi tiles arrive; t1/t2 after xlo.
        bq_t = ss_pool.tile([128, 1], F32, name="bq0", tag="bq", bufs=2)
        nc.sync.dma_start(bq_t[:], bq_d[0])
        # head-0 kt/v group 0 ahead of the xlo stream in the DMA queue
        g0_cache = load_group(0, 0)
        load_xlo()
        qT_t = qT_pool.tile([128, S], F16, name="qT0", tag="qT")
        psq_a = psq.tile([128, 512], F32, name="psq0_0", tag="psq")
        psq_b = kv_pool.tile([128, 512], F32, name="psq0_1", tag="kv")
        NP = DC // 2                    # 16 chunk pairs
        # xlo-dependent term last: t0 (xhi@Whi), t2 (xhi@Wlo), t1 (xlo@Whi)
        for pr in range(NP):
            q_mm(psq_a, wq0_hi, xbig_hi, pr, 0, pr == 0, False)
            q_mm(psq_b, wq0_hi, xbig_hi, pr, 1, pr == 0, False)
        for pr in range(NP):
            q_mm(psq_a, wq0_lo, xbig_hi, pr, 0, False, False)
            q_mm(psq_b, wq0_lo, xbig_hi, pr, 1, False, False)
        for pr in range(NP):
            q_mm(psq_a, wq0_hi, xbig_lo, pr, 0, False, pr == NP - 1)
            q_mm(psq_b, wq0_hi, xbig_lo, pr, 1, False, pr == NP - 1)
        q_half_add(qT_t, psq_a, 0, bq_t)
        q_half_add(qT_t, psq_b, 1, bq_t)

        if not ride_kv:
            kv_t = kv_pool.tile([128, 2 * HP], F32, name="kvT", tag="kv")
            for c in range(DC):
                for which in range(2):
                    for b in range(HP):
                        kv_mm(kv_t, c, which, b)
            kv_add(kv_t)

        wo_tiles = {}

        def mk_wo_dma(which, p_i, half):
            src = woh_d if which == 0 else wol_d
            nm = f"wo{'hl'[which]}{p_i}"

            def emit():
                t = wo_tiles.get((which, p_i))
                if t is None:
                    t = wo_pool.tile([128, 2 * D], E4, name=nm, tag=nm)
                    wo_tiles[(which, p_i)] = t
                nc.sync.dma_start(t[:, half * D:(half + 1) * D],
                                  src[p_i, :, half * D:(half + 1) * D])
            return emit

        def ct_ap(t, p_i, s_t):
            return t[p_i][:].rearrange("p (two s) -> p two s", two=2)[
                :, :, s_t * 128:(s_t + 1) * 128]

        def wo_ap(which, p_i, mg):
            return wo_tiles[(which, p_i)][:].rearrange(
                "p (two m) -> p two m", two=2)[:, :, mg * 512:(mg + 1) * 512]

        def o_mms(pso_t, p_i, s_t, mg, start, stop):
            """The 3 fp8 DoubleRow terms of pair p_i for out-tile (s_t, mg)."""
            mms = [(ct_ap(cthi, p_i, s_t), wo_ap(0, p_i, mg)),
                   (ct_ap(ctlo, p_i, s_t), wo_ap(0, p_i, mg)),
                   (ct_ap(cthi, p_i, s_t), wo_ap(1, p_i, mg))]
            for i, (lhs, rhs) in enumerate(mms):
                nc.tensor.matmul(pso_t[:], lhs, rhs,
                                 start=(start and i == 0),
                                 stop=(stop and i == 2), perf_mode=DR)

        o_staged = {}            # (s_t, mg) -> staged pair-0 partial (f16)
        N_STAGE = 24 if LC >= 24 else 0

        for h in range(HP):
            rides = [[] for _ in range(LC)]
            if h == HP - 2 and LC >= 16:
                # stream all Wo pair tiles during head 2's S loop (8 x 1MB)
                for idx, (which, p_i, hf) in enumerate(
                        (w, p, q) for w in range(2) for p in range(HP // 2)
                        for q in range(2)):
                    rides[2 + 3 * idx].append(mk_wo_dma(which, p_i, hf))
            if h == HP - 1 and N_STAGE:
                # ride the output projection's pair-0 (heads 0+1) terms in
                # head 3's PE slack; stage partials to SBUF f16
                o_tiles = [(s_t, mg) for s_t in range(S // 128)
                           for mg in range(D // 512)][:N_STAGE]
                ost = {}

                def mk_o(idx, item):
                    s_t, mg = item

                    def emit():
                        pool = psq if idx % 2 == 0 else kv_pool
                        ps_t = pool.tile([128, 512], F32, name=f"ops{idx}",
                                         tag="psq" if idx % 2 == 0 else "kv")
                        o_mms(ps_t, 0, s_t, mg, True, True)
                        sg = stage_pool.tile([128, 512], F16,
                                             name=f"sg{idx}", tag=f"sg{idx}")
                        nc.vector.tensor_copy(sg[:], ps_t[:])
                        o_staged[item] = sg
                    return emit

                for idx, item in enumerate(o_tiles):
                    rides[4 + idx].append(mk_o(idx, item))
            if h + 1 < HP and ride_q:
                bq1 = ss_pool.tile([128, 1], F32, name=f"bq{h+1}", tag="bq",
                                   bufs=2)
                nc.sync.dma_start(bq1[:], bq_d[h + 1])
                qT_next = qT_pool.tile([128, S], F16, name=f"qT{h+1}", tag="qT")
                state = {"wq": {}}

                # Ridden Q: sequential halves in the psq bank; per half,
                # 3 terms x 16 pair-mms. Whole-head wq DMAs ride slots 2/3.
                qwork = []
                for half in range(2):
                    for term in range(3):
                        for pr in range(NP):
                            qwork.append((half, term, pr))

                def mk_q(items, h1=h + 1, qn=qT_next, bqt=bq1, st=state):
                    def emit():
                        for half, term, pr in items:
                            if term == 0 and pr == 0:
                                st["psq"] = psq.tile(
                                    [128, 512], F32,
                                    name=f"psq{h1}_{half}", tag="psq")
                            wqt = st["wq"][0 if term < 2 else 1]
                            xb = xbig_lo if term == 1 else xbig_hi
                            last = (term, pr) == (2, NP - 1)
                            q_mm(st["psq"], wqt, xb, pr, half,
                                 term == 0 and pr == 0, last)
                            if last:
                                q_half_add(qn, st["psq"], half, bqt)
                    return emit

                def mk_wq(which, h1=h + 1, st=state):
                    def emit():
                        st["wq"][which] = emit_wq_dma(h1, which)
                    return emit

                # wq DMAs at slots 2/3; 96 mms over slots 4..31 (slots 0/1
                # ride-free so the first exps never wait on ride DMAs)
                rides[2].append(mk_wq(0))
                rides[3].append(mk_wq(1))
                per = max(1, -(-len(qwork) // min(LC - 6, 28)))
                for i in range(0, len(qwork), per):
                    rides[min(4 + i // per, LC - 1)].append(mk_q(qwork[i:i + per]))
            if h == 0 and ride_kv:
                kv_work = ([("mm", w, c, b) for c in range(DC)
                            for w in range(2) for b in range(HP)]
                           + [("add", 0, 0, 0)])
                kvstate = {}

                def kv_emit_one(item, st=kvstate):
                    kind, which, c, b = item
                    if kind == "add":
                        kv_add(st["kv"])
                        return
                    if which == 0 and c == 0 and b == 0:
                        st["kv"] = kv_pool.tile([128, 2 * HP], F32,
                                                name="kvT", tag="kv")
                    kv_mm(st["kv"], c, which, b)

                # spread over slots 2..npos_lt-2 (kvrow writes land at the
                # lt == npos_lt - 1 prefetch)
                n_slots = max(1, npos_lt - 3)
                per_kv = max(1, -(-len(kv_work) // n_slots))
                for k, item in enumerate(kv_work):
                    rides[min(2 + k // per_kv, LC - 1)].append(
                        (lambda it=item: kv_emit_one(it)))

            psc_t = psc.tile([128, S], F32, name=f"psc{h}", tag="psc")
            cur = g0_cache if h == 0 else load_group(h, 0)
            if not (h == 0 and ride_kv) and npos_g == 0 and npos_lt < LC:
                new_entry_writes(h, *cur)
            nxt = None
            pend = None              # lag-1 ctx: (lt, wt, vst)
            ps_by_lt = {}

            def emit_scores(lt, kt8):
                j = lt % 8
                ps = pss.tile([128, 1024], F32, name=f"ps_{h}_{lt}", tag="pss")
                ksl = kt8[:, j * 128:(j + 1) * 128]
                nc.tensor.matmul(ps[:, 0:512], ksl, qT_t[:, 0:512])
                nc.tensor.matmul(ps[:, 512:1024], ksl, qT_t[:, 512:1024])
                ps_by_lt[lt] = ps

            for lt in range(LC):
                g, j = lt // 8, lt % 8
                if j == 0 and g > 0:
                    cur = nxt
                if j == 0 and g + 1 < (LC + 7) // 8:
                    nxt = load_group(h, g + 1)
                    if not (h == 0 and ride_kv) and npos_g == g + 1:
                        new_entry_writes(h, *nxt)
                kt8, v8 = cur
                if lt == 0:
                    emit_scores(0, kt8)
                # prefetch scores for lt+1 ahead of ctx/rides so the ACT
                # engine's next exp input is ready a full tile early
                if lt + 1 < LC:
                    if h == 0 and ride_kv and lt + 1 == npos_lt:
                        # kvrow writes were emitted in rides at lt <= npos_lt-2
                        tgt = cur if (lt + 1) // 8 == g else nxt
                        new_entry_writes(h, *tgt)
                    emit_scores(lt + 1, kt8 if (lt + 1) // 8 == g else nxt[0])

                ps = ps_by_lt.pop(lt)

                for emit in rides[lt]:
                    emit()

                wt = wt_pool.tile([128, 1024], F32R, name=f"wt_{h}_{lt}", tag="wt")
                ssum = ss_pool.tile([128, 1], F32, name=f"ss_{h}_{lt}", tag="ssum")
                nc.scalar.activation(wt[:], ps[:], AF.Exp, scale=QSC, accum_out=ssum[:])
                rec = ss_pool.tile([128, 1], F32, name=f"rc_{h}_{lt}", tag="rec")
                nc.vector.reciprocal(rec[:], ssum[:])
                vst = vs_pool.tile([128, DK], F32R, name=f"vs{h}_{lt}", tag="vs")
                nc.vector.tensor_scalar_mul(vst[:], v8[:, j * 128:(j + 1) * 128], rec[:])

                if pend is not None:
                    plt, pwt, pvst = pend
                    nc.tensor.matmul(psc_t[:, 0:512], pvst[:], pwt[:, 0:512],
                                     start=(plt == 0), stop=False)
                    nc.tensor.matmul(psc_t[:, 512:1024], pvst[:], pwt[:, 512:1024],
                                     start=(plt == 0), stop=False)
                pend = (lt, wt, vst)
            plt, pwt, pvst = pend
            nc.tensor.matmul(psc_t[:, 0:512], pvst[:], pwt[:, 0:512],
                             start=(plt == 0), stop=True)
            nc.tensor.matmul(psc_t[:, 512:1024], pvst[:], pwt[:, 512:1024],
                             start=(plt == 0), stop=True)
            # quantize ctx*CS to e4m3 hi (+ lo residual) into the pair tiles
            p_i, s_i = h // 2, h % 2
            hi_ap = cthi[p_i][:, s_i * S:(s_i + 1) * S]
            lo_ap = ctlo[p_i][:, s_i * S:(s_i + 1) * S]
            nc.vector.tensor_scalar_mul(hi_ap, psc_t[:], float(CS))
            nc.vector.scalar_tensor_tensor(
                lo_ap, in0=psc_t[:], scalar=float(CS), in1=hi_ap,
                op0=ALU.mult, op1=ALU.subtract)
            if h + 1 < HP and not ride_q:
                bq1 = ss_pool.tile([128, 1], F32, name=f"bq{h+1}", tag="bq",
                                   bufs=2)
                nc.sync.dma_start(bq1[:], bq_d[h + 1])
                qT_next = qT_pool.tile([128, S], F16, name=f"qT{h+1}", tag="qT")
                wq_fb = [emit_wq_dma(h + 1, 0), emit_wq_dma(h + 1, 1)]
                for half in range(2):
                    psq_t = psq.tile([128, 512], F32,
                                     name=f"psq{h+1}_{half}", tag="psq")
                    for term in range(3):
                        for pr in range(NP):
                            wqt = wq_fb[0 if term < 2 else 1]
                            xb = xbig_lo if term == 1 else xbig_hi
                            q_mm(psq_t, wqt, xb, pr, half,
                                 term == 0 and pr == 0,
                                 term == 2 and pr == NP - 1)
                    q_half_add(qT_next, psq_t, half, bq1)
            if h + 1 < HP:
                qT_t = qT_next

        # release attention-phase pools before the output projection (LIFO)
        for p in (psc, pss, kv_pool, psq,
                  ss_pool, vs_pool, wt_pool, v_pool, kt_pool,
                  wkv_pool, wq_pool, small, qT_pool, xT_pool):
            p.release()

        # ---------- output projection: out[s, m] partial, 3-term fp8 ----------
        ob_pool = tc.alloc_tile_pool(name="obp", bufs=2)
        pso = tc.alloc_tile_pool(name="pso", bufs=4, space="PSUM")
        for which in range(2):
            for p_i in range(HP // 2):
                if (which, p_i) not in wo_tiles:   # short-seq fallback
                    for hf in range(2):
                        mk_wo_dma(which, p_i, hf)()

        n_fin = 0
        for s_t in range(S // 128):
            ob = ob_pool.tile([128, D], F16, name=f"ob{s_t}", tag="ob")
            for mg in range(D // 512):
                pso_t = pso.tile([128, 512], F32, name=f"po{s_t}_{mg}", tag="pso")
                sg = o_staged.get((s_t, mg))
                if sg is not None:
                    o_mms(pso_t, 1, s_t, mg, True, True)
                else:
                    o_mms(pso_t, 0, s_t, mg, True, False)
                    o_mms(pso_t, 1, s_t, mg, False, True)
                ob_sl = ob[:, mg * 512:(mg + 1) * 512]
                # staged adds on DVE; unstaged copies mostly on the idle ACT
                # (GPSIMD cannot access PSUM)
                if sg is not None:
                    nc.vector.tensor_add(ob_sl, pso_t[:], sg[:])
                elif n_fin % 3 != 2:
                    nc.scalar.activation(ob_sl, pso_t[:], AF.Copy)
                else:
                    nc.vector.tensor_copy(ob_sl, pso_t[:])
                n_fin += 1
            if s_t == S // 128 - 1:
                for q in range(8):
                    nc.sync.dma_start(
                        out_d[s_t * 128:(s_t + 1) * 128,
                              q * (D // 8):(q + 1) * (D // 8)],
                        ob[:, q * (D // 8):(q + 1) * (D // 8)])
            else:
                nc.sync.dma_start(out_d[s_t * 128:(s_t + 1) * 128, :], ob[:])
        for p in (pso, ob_pool, stage_pool, wo_pool, ct_pool):
            p.release()

    nc.compile()
    return nc


_CACHE = {}
LAST_EXEC_NS = None


def _split8(a):
    hi = np.asarray(a, E4NP)
    lo = np.asarray(a - hi.astype(np.float32), E4NP)
    return hi, lo


def kernel(x, k_cache, v_cache, Wq, bq, Wk, bk, Wv, bv, Wo, bo, pos):
    global LAST_EXEC_NS
    pos = int(pos)

    def f32(a):
        return np.ascontiguousarray(np.asarray(a), dtype=np.float32)

    x = f32(x)
    k_cache, v_cache = f32(k_cache), f32(v_cache)
    Wq, Wk, Wv, Wo = f32(Wq), f32(Wk), f32(Wv), f32(Wo)
    bq, bk, bv, bo = f32(bq), f32(bk), f32(bv), f32(bo)

    xT = x[0].T * np.float32(XS)                             # [D, S] * 16
    x8 = np.ascontiguousarray(
        xT.reshape(DC, 128, S).transpose(1, 0, 2).reshape(128, DC * S))
    xhi, xlo = _split8(x8)
    xl = np.ascontiguousarray(
        x[0, -1].reshape(DC, 128).T.astype(ml_dtypes.bfloat16))
    LGv = (pos + 1024) // 1024
    in_maps = []
    for i in range(NCORES):
        hs = slice(i * HP, (i + 1) * HP)
        wq_s = (Wq[hs] * np.float32(WS)).reshape(HP, DC, 128, DK).transpose(
            0, 2, 1, 3).reshape(HP, 128, DC * DK)
        wqh, wql = _split8(np.ascontiguousarray(wq_s))
        vp = np.zeros((HP, LGv * 1024, DK), np.float16)
        vp[:, :pos] = v_cache[hs, :pos].astype(np.float16)
        vg = np.ascontiguousarray(
            vp.reshape(HP, LGv, 8, 128, DK).transpose(0, 1, 3, 2, 4).reshape(
                HP, LGv, 128, 1024))
        # Wo pair layout: [pair, 128, 2*D]
        wo_s = Wo[i * HP * DK:(i + 1) * HP * DK] * np.float32(WS)  # [512, D]
        wo_p = wo_s.reshape(HP // 2, 2, 128, D).transpose(0, 2, 1, 3).reshape(
            HP // 2, 128, 2 * D)
        woh, wol = _split8(np.ascontiguousarray(wo_p))
        in_maps.append({
            "xhi": xhi, "xlo": xlo,
            "wqh": np.ascontiguousarray(wqh),
            "wql": np.ascontiguousarray(wql),
            "wkv": np.ascontiguousarray(np.concatenate([
                Wk[hs].transpose(1, 0, 2).reshape(D, HP * DK),
                Wv[hs].transpose(1, 0, 2).reshape(D, HP * DK)],
                axis=1).astype(ml_dtypes.bfloat16)),
            "xl": xl,
            "bq": np.ascontiguousarray(
                (bq[hs] * np.float32(XS * WS)).reshape(HP, DK, 1)),
            "bkv": np.ascontiguousarray(np.concatenate(
                [bk[hs].T, bv[hs].T], axis=1)),   # [128 k, which*HP+h]
            "kT": np.ascontiguousarray(
                k_cache[hs, :pos, :].transpose(0, 2, 1).astype(np.float16)),
            "v": vg,
            "woh": woh, "wol": wol,
        })

    if pos not in _CACHE:
        _CACHE[pos] = build(pos)
    nc = _CACHE[pos]

    res = run_bass_kernel_spmd(nc, in_maps, core_ids=list(range(NCORES)))
    LAST_EXEC_NS = res.exec_time_ns

    acc = np.zeros((S, D), np.float64)
    for r in res.results:
        acc += r["out"]
    out = (acc / (CS * WS) + bo.astype(np.float64)).astype(np.float32)
    return out[None]


# revision 39
# speedup vs baseline: 1.1211x; 1.0043x over previous
"""Trainium2 Bass kernel for CachedMultiHeadedAttention (tensor-parallel over heads).

Sharding: 8 cores x 4 heads. Each core computes Q projection + attention for
its 4 heads, then a partial output projection against its 512 rows of Wo.
Host sums the 8 partial outputs, divides by the fp8 scale product, adds bo.

v2: the Q and output projections run as 3-term e4m3 DoubleRow matmuls
(hi/lo splits, 0.5 cyc/row with 256-wide contraction = 4x fp16 rate):
  q  = xhi@Whi + xlo@Whi + xhi@Wlo        (x*16, Wq*256 host-split)
  out = cthi@Wohi + ctlo@Wohi + cthi@Wolo (ctx*32 device-split, Wo*256 host)
Dropped cross term contributes ~0.07%; measured end-to-end rel err ~1.7e-3.
Scores stay f16 (fp8 would put ~3.7% noise on the logits); softmax weights
and scaled V are f32r (exact, full PE rate). The exp scale folds away the
16*256 operand prescale (INV/4096); the host divides partials by 32*256.

Scheduling (cost-model-profiled):
  - Engine queues execute in order; the ACT-bound softmax loops carry "ride"
    work: the next head's 96 Q DoubleRow matmuls (+ wq hi/lo DMAs) and (in
    head 0) the kv_new projections are emitted inside the S loop, paced per
    l-tile, with ctx matmuls lagged one iteration behind the exp.
  - PSUM 8 banks: scores 2x[128,1024] (4) + ctx [128,1024] (2) + Q / kv_new
    accumulators (1+1).
  - Each dma_start costs ~625ns serialized HWDGE; DMAs are consolidated.
"""

import math

import numpy as np
import ml_dtypes

import concourse.bass as bass
import concourse.mybir as mybir
import concourse.tile as tile
from concourse import bacc
from concourse.bass_utils import run_bass_kernel_spmd

F32 = mybir.dt.float32
F32R = mybir.dt.float32r
BF16 = mybir.dt.bfloat16
F16 = mybir.dt.float16
E4 = mybir.dt.float8e4
AF = mybir.ActivationFunctionType
ALU = mybir.AluOpType
DR = mybir.MatmulPerfMode.DoubleRow
E4NP = ml_dtypes.float8_e4m3

H, D, DK, S = 32, 4096, 128, 1024
NCORES = 8
HP = H // NCORES          # heads per core
DC = D // 128             # contraction chunks for d_model
XS = 16.0                 # x prescale
WS = 256.0                # Wq / Wo prescale
CS = 32.0                 # ctx prescale (device-side split)


def build(pos: int):
    L = pos + 1
    LC = (L + 127) // 128          # number of 128-wide l tiles
    LG = (LC + 7) // 8             # l-tile groups of 8 (1024 l per group)
    INV = 1.0 / math.sqrt(DK)
    QSC = float(INV / (XS * WS))   # exp scale absorbing x/Wq prescale

    nc = bacc.Bacc("TRN2", target_bir_lowering=False, debug=False,
                   num_devices=NCORES)

    # pre-rearranged on host so every DMA descriptor is >= 512B contiguous:
    # x: [128, DC*S] with row p holding chunks c at cols c*S+s (x[c*128+p, s])
    # wq: [HP, 128, DC*DK] with row p holding chunk c at cols c*DK+k
    xhi_d = nc.dram_tensor("xhi", [128, DC * S], E4, kind="ExternalInput").ap()
    xlo_d = nc.dram_tensor("xlo", [128, DC * S], E4, kind="ExternalInput").ap()
    wqh_d = nc.dram_tensor("wqh", [HP, 128, DC * DK], E4, kind="ExternalInput").ap()
    wql_d = nc.dram_tensor("wql", [HP, 128, DC * DK], E4, kind="ExternalInput").ap()
    wkv_d = nc.dram_tensor("wkv", [D, 2 * HP * DK], BF16, kind="ExternalInput").ap()
    xl_d = nc.dram_tensor("xl", [128, DC], BF16, kind="ExternalInput").ap()
    bq_d = nc.dram_tensor("bq", [HP, DK, 1], F32, kind="ExternalInput").ap()
    bkv_d = nc.dram_tensor("bkv", [128, 2 * HP], F32, kind="ExternalInput").ap()
    kT_d = nc.dram_tensor("kT", [HP, DK, pos], F16, kind="ExternalInput").ap()
    # v pre-grouped on host: [h, g, p, i*DK+k] = v[h, g*1024+i*128+p, k],
    # zero-padded past pos (the new-entry row is overwritten on device)
    LGv = (pos + 1024) // 1024
    v_d = nc.dram_tensor("v", [HP, LGv, 128, 1024], F16, kind="ExternalInput").ap()
    # Wo pairs: [pair, 128, 2*D] with head-chunk 2p at cols 0:D, 2p+1 at D:2D
    woh_d = nc.dram_tensor("woh", [HP // 2, 128, 2 * D], E4, kind="ExternalInput").ap()
    wol_d = nc.dram_tensor("wol", [HP // 2, 128, 2 * D], E4, kind="ExternalInput").ap()
    out_d = nc.dram_tensor("out", [S, D], F16, kind="ExternalOutput").ap()

    with tile.TileContext(nc) as tc:
        # Pools are released LIFO; ct (quantized ctx) survives into the
        # output projection, so it sits at the bottom of the SBUF stack.
        ct_pool = tc.alloc_tile_pool(name="ctp", bufs=1)
        wo_pool = tc.alloc_tile_pool(name="wop", bufs=1)
        stage_pool = tc.alloc_tile_pool(name="stgp", bufs=1)
        xT_pool = tc.alloc_tile_pool(name="xT", bufs=1)
        qT_pool = tc.alloc_tile_pool(name="qT", bufs=2)
        small = tc.alloc_tile_pool(name="smallp", bufs=1)
        wq_pool = tc.alloc_tile_pool(name="wqp", bufs=2)
        wkv_pool = tc.alloc_tile_pool(name="wkvp", bufs=2)
        kt_pool = tc.alloc_tile_pool(name="ktp", bufs=3)
        v_pool = tc.alloc_tile_pool(name="vp", bufs=3)
        wt_pool = tc.alloc_tile_pool(name="wtp", bufs=4)
        vs_pool = tc.alloc_tile_pool(name="vsp", bufs=4)
        ss_pool = tc.alloc_tile_pool(name="ssp", bufs=8)

        psq = tc.alloc_tile_pool(name="psq", bufs=1, space="PSUM")
        kv_pool = tc.alloc_tile_pool(name="kvp", bufs=1, space="PSUM")
        pss = tc.alloc_tile_pool(name="pss", bufs=2, space="PSUM")
        psc = tc.alloc_tile_pool(name="psc", bufs=1, space="PSUM")

        # quantized-ctx pair tiles: [128, 2048] = heads (2p, 2p+1) side by side
        cthi = [ct_pool.tile([128, 2 * S], E4, name=f"cth{p}", tag=f"cth{p}")
                for p in range(HP // 2)]
        ctlo = [ct_pool.tile([128, 2 * S], E4, name=f"ctl{p}", tag=f"ctl{p}")
                for p in range(HP // 2)]

        # small constants first (tiny DMAs, ahead of the big streams)
        # kvrow layout: [128 k, which*HP + head] (transposed rank-1 results)
        kvrow = small.tile([128, 2 * HP], F16, name="kvrow", tag="kvrow")
        bkv_t = small.tile([128, 2 * HP], F32, name="bkvt", tag="bkvt")
        nc.sync.dma_start(bkv_t[:], bkv_d[:])
        xl_t = small.tile([128, DC], BF16, name="xlt", tag="xlt")
        nc.sync.dma_start(xl_t[:], xl_d[:])

        # resident x hi/lo tiles (8 big tiles of 4 chunks each per tensor),
        # hi tiles interleaved with head 0's Q weight groups so the first Q
        # matmuls start early; lo tiles follow.
        def emit_wq_dma(h, which):
            """One consolidated DMA for a whole head's Wq hi or lo tensor
            (16 separate dma_starts would cost 10us of serialized HWDGE)."""
            src = wqh_d if which == 0 else wql_d
            wqt = wq_pool.tile([128, DC * DK], E4,
                               name=f"wq{which}_{h}", tag=f"wq{which}")
            nc.sync.dma_start(wqt[:], src[h])
            return wqt

        wq0_hi = emit_wq_dma(0, 0)
        wq0_lo = emit_wq_dma(0, 1)
        xbig_hi, xbig_lo = [], []
        for gx in range(DC // 4):
            xt = xT_pool.tile([128, 4 * S], E4, name=f"xh{gx}", tag=f"xh{gx}")
            nc.sync.dma_start(xt[:], xhi_d[:, gx * 4 * S:(gx + 1) * 4 * S])
            xbig_hi.append(xt)

        def load_xlo():
            for gx in range(DC // 4):
                xt = xT_pool.tile([128, 4 * S], E4, name=f"xl{gx}", tag=f"xl{gx}")
                nc.sync.dma_start(xt[:], xlo_d[:, gx * 4 * S:(gx + 1) * 4 * S])
                xbig_lo.append(xt)

        def x_pair(xbig, pr, half):
            """rhs AP [128, 2, 512] for chunk pair (2pr, 2pr+1), s-half."""
            t = xbig[pr // 2]
            i = (pr % 2) * 2
            a3 = t[:, i * S:(i + 2) * S].rearrange("p (two s) -> p two s", two=2)
            return a3[:, :, half * 512:half * 512 + 512]

        def wq_pair(wqt, pr):
            """lhsT AP [128, 2, 128] for chunk pair (2pr, 2pr+1)."""
            return wqt[:, 2 * pr * DK:(2 * pr + 2) * DK].rearrange(
                "p (two k) -> p two k", two=2)

        def q_mm(psq_t, wqt, xbig, pr, half, start, stop):
            nc.tensor.matmul(psq_t[:], wq_pair(wqt, pr), x_pair(xbig, pr, half),
                             start=start, stop=stop, perf_mode=DR)

        def q_half_add(qT_t, psq_t, half, bq_t):
            nc.vector.tensor_scalar_add(qT_t[:, half * 512:(half + 1) * 512],
                                        psq_t[:], bq_t[:])

        kv_cur = {}

        def kv_mm(kv_t, c, which, b):
            """Transposed rank-1 update: out [128k, 1] per 128-wide k block
            (1 PE cycle each vs 512 for the [1, 512]-out orientation)."""
            if b == 0 and c % 4 == 0:
                wkvt = wkv_pool.tile([128, 4 * HP * DK], BF16,
                                     name=f"wkv{which}_{c}", tag="wkv")
                nc.sync.dma_start(
                    wkvt[:], wkv_d[c * 128:(c + 4) * 128,
                                   which * HP * DK:(which + 1) * HP * DK]
                    .rearrange("(i p) k -> p i k", p=128))
                kv_cur["t"] = wkvt
            wkvt = kv_cur["t"]
            nc.tensor.matmul(
                kv_t[:, which * HP + b:which * HP + b + 1],
                wkvt[:, (c % 4) * HP * DK + b * DK:
                     (c % 4) * HP * DK + (b + 1) * DK],
                xl_t[:, c:c + 1],
                start=(c == 0), stop=(c == DC - 1), skip_group_check=True)

        def kv_add(kv_t):
            nc.vector.tensor_add(kvrow[:], kv_t[:], bkv_t[:])

        def load_group(h, g):
            """Cache-only loads of l-group g."""
            g0 = g * 1024
            gl = min(1024, L - g0)            # valid l in group
            gc = max(0, min(1024, pos - g0))  # of which from cache
            kt8 = kt_pool.tile([128, 1024], F16, name=f"kt{h}_{g}", tag="kt")
            if gc > 0:
                nc.sync.dma_start(kt8[:, 0:gc], kT_d[h, :, g0:g0 + gc])
            if gl < 1024:
                nc.vector.memset(kt8[:, gl:1024], 0.0)
            v8 = v_pool.tile([128, 1024], F16, name=f"v{h}_{g}", tag="v")
            nc.sync.dma_start(v8[:], v_d[h, g])
            return kt8, v8

        def new_entry_writes(h, kt8, v8):
            gp = pos % 1024
            nc.sync.dma_start(kt8[:, gp:gp + 1], kvrow[:, h:h + 1])
            nc.sync.dma_start(
                v8[gp % 128:gp % 128 + 1, (gp // 128) * 128:(gp // 128 + 1) * 128],
                kvrow[:, HP + h:HP + h + 1])

        npos_g = pos // 1024            # l-group holding the new entry
        npos_lt = pos // 128            # l-tile index holding the new entry
        ride_kv = LC >= DC and npos_lt >= 8
        ride_q = LC >= DC

        # ---------- head 0 Q projection (phase A, DMA-paced) ----------
        # Both s-halves accumulate concurrently (pass B borrows the idle kv
        # bank) so the projection rides the x-arrival gaps. Term order:
        # t0 = xhi (x)Whi per pair as xhi tiles arrive; t1/t2 after xlo.
        bq_t = ss_pool.tile([128, 1], F32, name="bq0", tag="bq", bufs=2)
        nc.sync.dma_start(bq_t[:], bq_d[0])
        # head-0 kt/v group 0 ahead of the xlo stream in the DMA queue
        g0_cache = load_group(0, 0)
        load_xlo()
        qT_t = qT_pool.tile([128, S], F16, name="qT0", tag="qT")
        psq_a = psq.tile([128, 512], F32, name="psq0_0", tag="psq")
        psq_b = kv_pool.tile([128, 512], F32, name="psq0_1", tag="kv")
        NP = DC // 2                    # 16 chunk pairs
        # xlo-dependent term last: t0 (xhi@Whi), t2 (xhi@Wlo), t1 (xlo@Whi)
        for pr in range(NP):
            q_mm(psq_a, wq0_hi, xbig_hi, pr, 0, pr == 0, False)
            q_mm(psq_b, wq0_hi, xbig_hi, pr, 1, pr == 0, False)
        for pr in range(NP):
            q_mm(psq_a, wq0_lo, xbig_hi, pr, 0, False, False)
            q_mm(psq_b, wq0_lo, xbig_hi, pr, 1, False, False)
        for pr in range(NP):
            q_mm(psq_a, wq0_hi, xbig_lo, pr, 0, False, pr == NP - 1)
            q_mm(psq_b, wq0_hi, xbig_lo, pr, 1, False, pr == NP - 1)
        q_half_add(qT_t, psq_a, 0, bq_t)
        q_half_add(qT_t, psq_b, 1, bq_t)

        if not ride_kv:
            kv_t = kv_pool.tile([128, 2 * HP], F32, name="kvT", tag="kv")
            for c in range(DC):
                for which in range(2):
                    for b in range(HP):
                        kv_mm(kv_t, c, which, b)
            kv_add(kv_t)

        wo_tiles = {}

        def mk_wo_dma(which, p_i, half):
            src = woh_d if which == 0 else wol_d
            nm = f"wo{'hl'[which]}{p_i}"

            def emit():
                t = wo_tiles.get((which, p_i))
                if t is None:
                    t = wo_pool.tile([128, 2 * D], E4, name=nm, tag=nm)
                    wo_tiles[(which, p_i)] = t
                nc.sync.dma_start(t[:, half * D:(half + 1) * D],
                                  src[p_i, :, half * D:(half + 1) * D])
            return emit

        def ct_ap(t, p_i, s_t):
            return t[p_i][:].rearrange("p (two s) -> p two s", two=2)[
                :, :, s_t * 128:(s_t + 1) * 128]

        def wo_ap(which, p_i, mg):
            return wo_tiles[(which, p_i)][:].rearrange(
                "p (two m) -> p two m", two=2)[:, :, mg * 512:(mg + 1) * 512]

        def o_mms(pso_t, p_i, s_t, mg, start, stop):
            """The 3 fp8 DoubleRow terms of pair p_i for out-tile (s_t, mg)."""
            mms = [(ct_ap(cthi, p_i, s_t), wo_ap(0, p_i, mg)),
                   (ct_ap(ctlo, p_i, s_t), wo_ap(0, p_i, mg)),
                   (ct_ap(cthi, p_i, s_t), wo_ap(1, p_i, mg))]
            for i, (lhs, rhs) in enumerate(mms):
                nc.tensor.matmul(pso_t[:], lhs, rhs,
                                 start=(start and i == 0),
                                 stop=(stop and i == 2), perf_mode=DR)

        o_staged = {}            # (s_t, mg) -> staged pair-0 partial (f16)
        N_STAGE = 0

        for h in range(HP):
            rides = [[] for _ in range(LC)]
            if h == HP - 2 and LC >= 16:
                # stream all Wo pair tiles during head 2's S loop (8 x 1MB)
                for idx, (which, p_i, hf) in enumerate(
                        (w, p, q) for w in range(2) for p in range(HP // 2)
                        for q in range(2)):
                    rides[2 + 3 * idx].append(mk_wo_dma(which, p_i, hf))
            if h == HP - 1 and N_STAGE:
                # ride the output projection's pair-0 (heads 0+1) terms in
                # head 3's PE slack; stage partials to SBUF f16
                o_tiles = [(s_t, mg) for s_t in range(S // 128)
                           for mg in range(D // 512)][:N_STAGE]
                ost = {}

                def mk_o(idx, item):
                    s_t, mg = item

                    def emit():
                        pool = psq if idx % 2 == 0 else kv_pool
                        ps_t = pool.tile([128, 512], F32, name=f"ops{idx}",
                                         tag="psq" if idx % 2 == 0 else "kv")
                        o_mms(ps_t, 0, s_t, mg, True, True)
                        sg = stage_pool.tile([128, 512], F16,
                                             name=f"sg{idx}", tag=f"sg{idx}")
                        nc.vector.tensor_copy(sg[:], ps_t[:])
                        o_staged[item] = sg
                    return emit

                for idx, item in enumerate(o_tiles):
                    rides[4 + idx].append(mk_o(idx, item))
            if h + 1 < HP and ride_q:
                bq1 = ss_pool.tile([128, 1], F32, name=f"bq{h+1}", tag="bq",
                                   bufs=2)
                nc.sync.dma_start(bq1[:], bq_d[h + 1])
                qT_next = qT_pool.tile([128, S], F16, name=f"qT{h+1}", tag="qT")
                state = {"wq": {}}

                # Ridden Q: sequential halves in the psq bank; per half,
                # 3 terms x 16 pair-mms. Whole-head wq DMAs ride slots 2/3.
                qwork = []
                for half in range(2):
                    for term in range(3):
                        for pr in range(NP):
                            qwork.append((half, term, pr))

                def mk_q(items, h1=h + 1, qn=qT_next, bqt=bq1, st=state):
                    def emit():
                        for half, term, pr in items:
                            if term == 0 and pr == 0:
                                st["psq"] = psq.tile(
                                    [128, 512], F32,
                                    name=f"psq{h1}_{half}", tag="psq")
                            wqt = st["wq"][0 if term < 2 else 1]
                            xb = xbig_lo if term == 1 else xbig_hi
                            last = (term, pr) == (2, NP - 1)
                            q_mm(st["psq"], wqt, xb, pr, half,
                                 term == 0 and pr == 0, last)
                            if last:
                                q_half_add(qn, st["psq"], half, bqt)
                    return emit

                def mk_wq(which, h1=h + 1, st=state):
                    def emit():
                        st["wq"][which] = emit_wq_dma(h1, which)
                    return emit

                # wq DMAs at slots 2/3; 96 mms over slots 4..31 (slots 0/1
                # ride-free so the first exps never wait on ride DMAs)
                rides[2].append(mk_wq(0))
                rides[3].append(mk_wq(1))
                per = max(1, -(-len(qwork) // min(LC - 6, 28)))
                for i in range(0, len(qwork), per):
                    rides[min(4 + i // per, LC - 1)].append(mk_q(qwork[i:i + per]))
            if h == 0 and ride_kv:
                kv_work = ([("mm", w, c, b) for c in range(DC)
                            for w in range(2) for b in range(HP)]
                           + [("add", 0, 0, 0)])
                kvstate = {}

                def kv_emit_one(item, st=kvstate):
                    kind, which, c, b = item
                    if kind == "add":
                        kv_add(st["kv"])
                        return
                    if which == 0 and c == 0 and b == 0:
                        st["kv"] = kv_pool.tile([128, 2 * HP], F32,
                                                name="kvT", tag="kv")
                    kv_mm(st["kv"], c, which, b)

                # spread over slots 2..npos_lt-2 (kvrow writes land at the
                # lt == npos_lt - 1 prefetch)
                n_slots = max(1, npos_lt - 3)
                per_kv = max(1, -(-len(kv_work) // n_slots))
                for k, item in enumerate(kv_work):
                    rides[min(2 + k // per_kv, LC - 1)].append(
                        (lambda it=item: kv_emit_one(it)))

            psc_t = psc.tile([128, S], F32, name=f"psc{h}", tag="psc")
            cur = g0_cache if h == 0 else load_group(h, 0)
            if not (h == 0 and ride_kv) and npos_g == 0 and npos_lt < LC:
                new_entry_writes(h, *cur)
            nxt = None
            pend = None              # lag-1 ctx: (lt, wt, vst)
            ps_by_lt = {}

            def emit_scores(lt, kt8):
                j = lt % 8
                ps = pss.tile([128, 1024], F32, name=f"ps_{h}_{lt}", tag="pss")
                ksl = kt8[:, j * 128:(j + 1) * 128]
                nc.tensor.matmul(ps[:, 0:512], ksl, qT_t[:, 0:512])
                nc.tensor.matmul(ps[:, 512:1024], ksl, qT_t[:, 512:1024])
                ps_by_lt[lt] = ps

            for lt in range(LC):
                g, j = lt // 8, lt % 8
                if j == 0 and g > 0:
                    cur = nxt
                if j == 0 and g + 1 < (LC + 7) // 8:
                    nxt = load_group(h, g + 1)
                    if not (h == 0 and ride_kv) and npos_g == g + 1:
                        new_entry_writes(h, *nxt)
                kt8, v8 = cur
                if lt == 0:
                    emit_scores(0, kt8)
                # prefetch scores for lt+1 ahead of ctx/rides so the ACT
                # engine's next exp input is ready a full tile early
                if lt + 1 < LC:
                    if h == 0 and ride_kv and lt + 1 == npos_lt:
                        # kvrow writes were emitted in rides at lt <= npos_lt-2
                        tgt = cur if (lt + 1) // 8 == g else nxt
                        new_entry_writes(h, *tgt)
                    emit_scores(lt + 1, kt8 if (lt + 1) // 8 == g else nxt[0])

                ps = ps_by_lt.pop(lt)

                for emit in rides[lt]:
                    emit()

                wt = wt_pool.tile([128, 1024], F32R, name=f"wt_{h}_{lt}", tag="wt")
                ssum = ss_pool.tile([128, 1], F32, name=f"ss_{h}_{lt}", tag="ssum")
                nc.scalar.activation(wt[:], ps[:], AF.Exp, scale=QSC, accum_out=ssum[:])
                rec = ss_pool.tile([128, 1], F32, name=f"rc_{h}_{lt}", tag="rec")
                nc.vector.reciprocal(rec[:], ssum[:])
                vst = vs_pool.tile([128, DK], F32R, name=f"vs{h}_{lt}", tag="vs")
                nc.vector.tensor_scalar_mul(vst[:], v8[:, j * 128:(j + 1) * 128], rec[:])

                if pend is not None:
                    plt, pwt, pvst = pend
                    nc.tensor.matmul(psc_t[:, 0:512], pvst[:], pwt[:, 0:512],
                                     start=(plt == 0), stop=False)
                    nc.tensor.matmul(psc_t[:, 512:1024], pvst[:], pwt[:, 512:1024],
                                     start=(plt == 0), stop=False)
                pend = (lt, wt, vst)
            plt, pwt, pvst = pend
            nc.tensor.matmul(psc_t[:, 0:512], pvst[:], pwt[:, 0:512],
                             start=(plt == 0), stop=True)
            nc.tensor.matmul(psc_t[:, 512:1024], pvst[:], pwt[:, 512:1024],
                             start=(plt == 0), stop=True)
            # quantize ctx*CS to e4m3 hi (+ lo residual) into the pair tiles
            p_i, s_i = h // 2, h % 2
            hi_ap = cthi[p_i][:, s_i * S:(s_i + 1) * S]
            lo_ap = ctlo[p_i][:, s_i * S:(s_i + 1) * S]
            nc.vector.tensor_scalar_mul(hi_ap, psc_t[:], float(CS))
            nc.vector.scalar_tensor_tensor(
                lo_ap, in0=psc_t[:], scalar=float(CS), in1=hi_ap,
                op0=ALU.mult, op1=ALU.subtract)
            if h + 1 < HP and not ride_q:
                bq1 = ss_pool.tile([128, 1], F32, name=f"bq{h+1}", tag="bq",
                                   bufs=2)
                nc.sync.dma_start(bq1[:], bq_d[h + 1])
                qT_next = qT_pool.tile([128, S], F16, name=f"qT{h+1}", tag="qT")
                wq_fb = [emit_wq_dma(h + 1, 0), emit_wq_dma(h + 1, 1)]
                for half in range(2):
                    psq_t = psq.tile([128, 512], F32,
                                     name=f"psq{h+1}_{half}", tag="psq")
                    for term in range(3):
                        for pr in range(NP):
                            wqt = wq_fb[0 if term < 2 else 1]
                            xb = xbig_lo if term == 1 else xbig_hi
                            q_mm(psq_t, wqt, xb, pr, half,
                                 term == 0 and pr == 0,
                                 term == 2 and pr == NP - 1)
                    q_half_add(qT_next, psq_t, half, bq1)
            if h + 1 < HP:
                qT_t = qT_next

        # release attention-phase pools before the output projection (LIFO)
        for p in (psc, pss, kv_pool, psq,
                  ss_pool, vs_pool, wt_pool, v_pool, kt_pool,
                  wkv_pool, wq_pool, small, qT_pool, xT_pool):
            p.release()

        # ---------- output projection: out[s, m] partial, 3-term fp8 ----------
        ob_pool = tc.alloc_tile_pool(name="obp", bufs=2)
        pso = tc.alloc_tile_pool(name="pso", bufs=4, space="PSUM")
        for which in range(2):
            for p_i in range(HP // 2):
                if (which, p_i) not in wo_tiles:   # short-seq fallback
                    for hf in range(2):
                        mk_wo_dma(which, p_i, hf)()

        n_fin = 0
        for s_t in range(S // 128):
            ob = ob_pool.tile([128, D], F16, name=f"ob{s_t}", tag="ob")
            for mg in range(D // 512):
                pso_t = pso.tile([128, 512], F32, name=f"po{s_t}_{mg}", tag="pso")
                sg = o_staged.get((s_t, mg))
                if sg is not None:
                    o_mms(pso_t, 1, s_t, mg, True, True)
                else:
                    o_mms(pso_t, 0, s_t, mg, True, False)
                    o_mms(pso_t, 1, s_t, mg, False, True)
                ob_sl = ob[:, mg * 512:(mg + 1) * 512]
                # staged adds on DVE; unstaged copies mostly on the idle ACT
                # (GPSIMD cannot access PSUM)
                if sg is not None:
                    nc.vector.tensor_add(ob_sl, pso_t[:], sg[:])
                elif n_fin % 3 != 2:
                    nc.scalar.activation(ob_sl, pso_t[:], AF.Copy)
                else:
                    nc.vector.tensor_copy(ob_sl, pso_t[:])
                n_fin += 1
            if s_t == S // 128 - 1:
                for q in range(8):
                    nc.sync.dma_start(
                        out_d[s_t * 128:(s_t + 1) * 128,
                              q * (D // 8):(q + 1) * (D // 8)],
                        ob[:, q * (D // 8):(q + 1) * (D // 8)])
            else:
                nc.sync.dma_start(out_d[s_t * 128:(s_t + 1) * 128, :], ob[:])
        for p in (pso, ob_pool, stage_pool, wo_pool, ct_pool):
            p.release()

    nc.compile()
    return nc


_CACHE = {}
LAST_EXEC_NS = None


def _split8(a):
    hi = np.asarray(a, E4NP)
    lo = np.asarray(a - hi.astype(np.float32), E4NP)
    return hi, lo


def kernel(x, k_cache, v_cache, Wq, bq, Wk, bk, Wv, bv, Wo, bo, pos):
    global LAST_EXEC_NS
    pos = int(pos)

    def f32(a):
        return np.ascontiguousarray(np.asarray(a), dtype=np.float32)

    x = f32(x)
    k_cache, v_cache = f32(k_cache), f32(v_cache)
    Wq, Wk, Wv, Wo = f32(Wq), f32(Wk), f32(Wv), f32(Wo)
    bq, bk, bv, bo = f32(bq), f32(bk), f32(bv), f32(bo)

    xT = x[0].T * np.float32(XS)                             # [D, S] * 16
    x8 = np.ascontiguousarray(
        xT.reshape(DC, 128, S).transpose(1, 0, 2).reshape(128, DC * S))
    xhi, xlo = _split8(x8)
    xl = np.ascontiguousarray(
        x[0, -1].reshape(DC, 128).T.astype(ml_dtypes.bfloat16))
    LGv = (pos + 1024) // 1024
    in_maps = []
    for i in range(NCORES):
        hs = slice(i * HP, (i + 1) * HP)
        wq_s = (Wq[hs] * np.float32(WS)).reshape(HP, DC, 128, DK).transpose(
            0, 2, 1, 3).reshape(HP, 128, DC * DK)
        wqh, wql = _split8(np.ascontiguousarray(wq_s))
        vp = np.zeros((HP, LGv * 1024, DK), np.float16)
        vp[:, :pos] = v_cache[hs, :pos].astype(np.float16)
        vg = np.ascontiguousarray(
            vp.reshape(HP, LGv, 8, 128, DK).transpose(0, 1, 3, 2, 4).reshape(
                HP, LGv, 128, 1024))
        # Wo pair layout: [pair, 128, 2*D]
        wo_s = Wo[i * HP * DK:(i + 1) * HP * DK] * np.float32(WS)  # [512, D]
        wo_p = wo_s.reshape(HP // 2, 2, 128, D).transpose(0, 2, 1, 3).reshape(
            HP // 2, 128, 2 * D)
        woh, wol = _split8(np.ascontiguousarray(wo_p))
        in_maps.append({
            "xhi": xhi, "xlo": xlo,
            "wqh": np.ascontiguousarray(wqh),
            "wql": np.ascontiguousarray(wql),
            "wkv": np.ascontiguousarray(np.concatenate([
                Wk[hs].transpose(1, 0, 2).reshape(D, HP * DK),
                Wv[hs].transpose(1, 0, 2).reshape(D, HP * DK)],
                axis=1).astype(ml_dtypes.bfloat16)),
            "xl": xl,
            "bq": np.ascontiguousarray(
                (bq[hs] * np.float32(XS * WS)).reshape(HP, DK, 1)),
            "bkv": np.ascontiguousarray(np.concatenate(
                [bk[hs].T, bv[hs].T], axis=1)),   # [128 k, which*HP+h]
            "kT": np.ascontiguousarray(
                k_cache[hs, :pos, :].transpose(0, 2, 1).astype(np.float16)),
            "v": vg,
            "woh": woh, "wol": wol,
        })

    if pos not in _CACHE:
        _CACHE[pos] = build(pos)
    nc = _CACHE[pos]

    res = run_bass_kernel_spmd(nc, in_maps, core_ids=list(range(NCORES)))
    LAST_EXEC_NS = res.exec_time_ns

    acc = np.zeros((S, D), np.float64)
    for r in res.results:
        acc += r["out"]
    out = (acc / (CS * WS) + bo.astype(np.float64)).astype(np.float32)
    return out[None]
